# revision 20
# baseline (speedup 1.0000x reference)
"""ConvLSTM2D block (ConvLSTM -> BatchNorm -> MaxPool2x2) on 8 Trainium2 cores.

Problem (hardcoded): x [B=4, T=16, H=64, W=64, Cin=64], ConvLSTM2D with
3x3 kernels, C=64 channels, keras gate order (i, f, g, o), hard_sigmoid
recurrent activation, tanh activation, inference BatchNorm, spatial 2x2
max pool -> out [4, 16, 32, 32, 64] fp32.

Sharding: 8 shards = batch(4) x H-halves(2). Each core computes a 48-row
slice of one sample's recurrence; the 16-row overlap is recomputed
redundantly (a 3x3 recurrent conv corrupts one boundary row per step).
Bottom-half shards are fed ROW-FLIPPED data (and row-flipped conv taps)
so that every core's owned rows are local rows 0:32 - this makes the
per-step compute window shrinkable in the same SPMD program: step t only
needs rows 0:48-t correct, so t>=8 runs 5 blocks instead of 6, and the
pool/BN/store stage covers only rows 0:32.

Layout (activations bf16, PSUM accumulate fp32):
- plane [128, 50, 66] bf16: partitions 0:64 = h_t (the LSTM pointwise
  lands on partitions 0:64, writing h straight into the next plane with a
  strided DVE op), 64:128 = x_t.
- stationary tap j: rows 0:64 = U tap, 64:128 = W tap; gate columns
  [f,i | o,g]; f,i (o) columns pre-scaled by 0.2 (-0.2) so hard_sigmoid
  needs no multiply. Per 512-px block two M=128 PSUM groups: [f;i], [o;g].
- pointwise: DVE clips f,i and forms the products/c update; ACT does the
  g and c tanhs plus the o hard_sigmoid as a Relu chain (g first - it is
  on the h critical path); the only cross-partition move is the i*g fold
  (one 64-part DMA per block, issued on the Pool engine's DGE).
- a few warmup matmuls at t=0 ramp the PE p-state while x DMAs land.
"""
import sys
sys.path.insert(0, '/opt/trn_rl_repo')

import numpy as np
import ml_dtypes

import bass_rust
import concourse.bass as bass
import concourse.tile as tile
from concourse import mybir
from concourse.bass_utils import run_bass_kernel_spmd

F32 = mybir.dt.float32
BF16 = mybir.dt.bfloat16
ALU = mybir.AluOpType
ACTF = mybir.ActivationFunctionType

B, T, H, W, C = 4, 16, 64, 64, 64
BN_EPS = 1e-3
HS = 48           # rows per shard
RP, CP = 50, 66   # padded plane rows/cols

_cached = None


def _split_multi_waits(nc, limit=1):
    """walrus here encodes at most one sem-wait per instruction; move excess
    waits onto nops inserted before the instruction on the same engine."""
    cnt = 0
    for fn in nc.m.functions:
        for bb in fn.blocks:
            out, changed = [], False
            for inst in bb.instructions:
                si = inst.sync_info
                waits = list(si.on_wait) if (si and si.on_wait) else []
                if len(waits) > limit:
                    changed = True
                    extra, keep = waits[:-limit], waits[-limit:]
                    for i in range(0, len(extra), limit):
                        cnt += 1
                        nop = mybir.InstNoOp(name=f"I-wsplit-{cnt}", engine=inst.engine)
                        nop.sync_info = bass_rust.SyncInfo(
                            on_wait=extra[i:i + limit], on_update=[])
                        out.append(nop)
                    si.on_wait = keep
                out.append(inst)
            if changed:
                bb.instructions = out


def _rows(t):
    # step t needs h_{t+1} rows 0:48-t correct; the last step only feeds
    # the pool (rows 0:32). The final block of a step is partial - matmul
    # cost scales with N, so computing exactly the needed rows is free.
    return 32 if t == 15 else 48 - t


def _build():
    nc = bass.Bass()
    x_d = nc.dram_tensor("xc", [T, C, HS, CP], BF16, kind="ExternalInput")
    w_d = nc.dram_tensor("wstk", [128, 9, 256], BF16, kind="ExternalInput")
    cn_d = nc.dram_tensor("consts", [128, 4], F32, kind="ExternalInput")
    z_d = nc.dram_tensor("zeros", [128, RP, CP], BF16, kind="ExternalInput")
    f_d = nc.dram_tensor("foldw", [128, 64], BF16, kind="ExternalInput")
    y_d = nc.dram_tensor("yout", [T, C, 16 * 32], F32, kind="ExternalOutput")

    with tile.TileContext(nc) as tc:
        with (
            tc.tile_pool(name="state", bufs=1) as st,
            tc.tile_pool(name="scr", bufs=6) as sc,
            tc.tile_pool(name="pool_scr", bufs=2) as pscr,
            tc.tile_pool(name="psum", bufs=3, space="PSUM") as pp,
            tc.tile_pool(name="psum_w", bufs=1, space="PSUM") as ppw,
            tc.tile_pool(name="psum_c", bufs=1, space="PSUM") as ppc,
        ):
            wsb = st.tile([128, 9, 256], BF16, tag="wsb")
            nc.sync.dma_start(out=wsb, in_=w_d[:, :, :])
            cons = st.tile([128, 4], F32, tag="cons")
            nc.sync.dma_start(out=cons, in_=cn_d[:, :])
            fw = st.tile([128, 64], BF16, tag="fw")
            nc.sync.dma_start(out=fw, in_=f_d[:, :])
            b_fi = cons[:, 0:1]       # 0.2*b_f + 0.5 ; 0.2*b_i + 0.5
            b_o = cons[0:64, 1:2]     # 0.5 - 0.2*b_o
            b_g = cons[64:128, 1:2]   # b_g
            bns = cons[0:64, 2:3]     # BN scale
            bnb = cons[0:64, 3:4]     # BN bias

            # planes: partitions 0:63 = h_t, 64:127 = x_t, double buffered
            xh = [st.tile([128, RP * CP], BF16, tag=f"xh{i}", name=f"xh{i}")
                  for i in range(2)]
            cg = st.tile([128, HS * W], BF16, tag="cg")
            nc.vector.memset(cg, 0.0)

            # ramp the PE p-state while the first x tile is in flight
            wflat = wsb.rearrange("p a b -> p (a b)")
            wps = ppw.tile([128, 512], F32, tag="warm")
            for _ in range(8):
                nc.tensor.matmul(wps, wsb[:, 0, 0:128], wflat[:, 0:512],
                                 start=True, stop=True)
            aw = sc.tile([64, 1], F32, tag="actwarm")
            nc.scalar.activation(aw, cons[0:64, 0:1], ACTF.Tanh)
            gw = sc.tile([64, 4], F32, tag="dgewarm")
            nc.gpsimd.dma_start(out=gw, in_=cons[0:64, :])

            def pv(tns):
                return tns.rearrange("p (r c) -> p r c", r=RP)

            for tns in xh:
                p = pv(tns)
                nc.gpsimd.dma_start(out=tns[0:64, :],
                                    in_=z_d[0:64, :, :])
                nc.gpsimd.dma_start(out=p[64:128, 0:1, :],
                                    in_=z_d[64:128, 0:1, :])
                nc.gpsimd.dma_start(out=p[64:128, 49:50, :],
                                    in_=z_d[64:128, 49:50, :])
            nc.sync.dma_start(out=pv(xh[0])[64:128, 1:49, :],
                              in_=x_d[0, :, :, :])

            for t in range(T):
                cur = pv(xh[t % 2])
                nxt = pv(xh[(t + 1) % 2])
                if t + 1 < T:
                    nc.sync.dma_start(out=nxt[64:128, 1:49, :],
                                      in_=x_d[t + 1, :, :, :])

                rows = _rows(t)
                nb = -(-rows // 8)
                for blk in range(nb):
                    r0 = blk * 8
                    rr = min(8, rows - r0)   # last block may be partial
                    npx = rr * 64
                    fs = slice(blk * 512, blk * 512 + npx)
                    pst = []
                    for mh in range(2):
                        ps = pp.tile([128, 512], F32, tag=f"ps{mh}",
                                     name=f"ps_{t}_{blk}_{mh}")
                        pst.append(ps)
                        for j in range(9):
                            a0, b0 = j // 3, j % 3
                            rhs = cur[:, r0 + a0:r0 + a0 + rr, b0:b0 + 64]
                            nc.tensor.matmul(
                                ps[:, 0:npx],
                                wsb[:, j, mh * 128:(mh + 1) * 128], rhs,
                                start=(j == 0), stop=(j == 8))
                    ps0, ps1 = pst

                    # f,i: clip(z' + b', 0, 1) on DVE (scale folded in W)
                    fi2 = sc.tile([128, 512], BF16, tag="fi2")
                    nc.vector.tensor_scalar(fi2[:, 0:npx], ps0[:, 0:npx],
                                            b_fi, 0.0, ALU.add, ALU.max)
                    nc.vector.tensor_scalar_min(fi2[:, 0:npx], fi2[:, 0:npx],
                                                1.0)
                    # g first - it gates the h critical path
                    nc.scalar.activation(cg[64:128, fs], ps1[64:128, 0:npx],
                                         ACTF.Tanh, bias=b_g, scale=1.0)
                    # o: hard_sigmoid via Relu chain on ACT
                    # (o columns folded with -0.2): relu(1 - relu(z''+b''))
                    oo1 = sc.tile([64, 512], BF16, tag="oo1")
                    nc.scalar.activation(oo1[:, 0:npx], ps1[0:64, 0:npx],
                                         ACTF.Relu, bias=b_o, scale=1.0)
                    oo = sc.tile([64, 512], BF16, tag="oo")
                    nc.scalar.activation(oo[:, 0:npx], oo1[:, 0:npx],
                                         ACTF.Relu, bias=1.0, scale=-1.0)
                    # pr = [f*c ; i*g]; fold i*g down via the Pool DGE -
                    # except the very last block, where the PE sits idle:
                    # fold via matmul against [I;I] straight into PSUM and
                    # skip the c writeback (c_16 is never read).
                    pr = sc.tile([128, 512], BF16, tag="pr")
                    nc.vector.tensor_tensor(pr[:, 0:npx], fi2[:, 0:npx],
                                            cg[:, fs], ALU.mult)
                    tct = sc.tile([64, 512], BF16, tag="tct")
                    if t == 15 and blk == nb - 1:
                        cps = ppc.tile([64, 512], F32, tag="cps")
                        nc.tensor.matmul(cps[:, 0:npx], fw, pr[:, 0:npx],
                                         start=True, stop=True)
                        nc.scalar.activation(tct[:, 0:npx], cps[:, 0:npx],
                                             ACTF.Tanh)
                    else:
                        prm = sc.tile([64, 512], BF16, tag="prm")
                        nc.gpsimd.dma_start(out=prm[:, 0:npx],
                                            in_=pr[64:128, 0:npx])
                        nc.vector.tensor_tensor(cg[0:64, fs], pr[0:64, 0:npx],
                                                prm[:, 0:npx], ALU.add)
                        nc.scalar.activation(tct[:, 0:npx], cg[0:64, fs],
                                             ACTF.Tanh)
                    # h = o * tanh(c), written straight into next plane
                    nc.vector.tensor_tensor(
                        nxt[0:64, r0 + 1:r0 + 1 + rr, 1:65],
                        tct[:, 0:npx].rearrange("p (r c) -> p r c", r=rr),
                        oo[:, 0:npx].rearrange("p (r c) -> p r c", r=rr),
                        ALU.mult)

                # BN + 2x2 max pool on h_{t+1}, owned rows 0:32 only
                s1 = pscr.tile([64, 32, 32], BF16, tag="s1")
                nc.vector.tensor_tensor(s1, nxt[0:64, 1:33, 1:65:2],
                                        nxt[0:64, 1:33, 2:66:2], ALU.max)
                s2 = pscr.tile([64, 16, 32], BF16, tag="s2")
                nc.vector.tensor_tensor(s2, s1[:, 0:32:2, :], s1[:, 1:32:2, :],
                                        ALU.max)
                yt = pscr.tile([64, 16 * 32], F32, tag="yt")
                nc.scalar.activation(yt, s2.rearrange("p a b -> p (a b)"),
                                     ACTF.Identity, bias=bnb, scale=bns)
                nc.sync.dma_start(out=y_d[t, :, :], in_=yt)

    _split_multi_waits(nc)
    return nc


def _get_nc():
    global _cached
    if _cached is None:
        _cached = _build()
    return _cached


def _prep_inputs(input_tensor, W_, U, b, gamma, beta, moving_mean, moving_var):
    x = np.asarray(input_tensor, np.float32)
    W_ = np.asarray(W_, np.float32)
    U = np.asarray(U, np.float32)
    b = np.asarray(b, np.float32)
    gamma = np.asarray(gamma, np.float32)
    beta = np.asarray(beta, np.float32)
    moving_mean = np.asarray(moving_mean, np.float32)
    moving_var = np.asarray(moving_var, np.float32)

    # Cout reorder (i,f,g,o) -> (f,i,o,g); fold hard_sigmoid scales into
    # the f,i (+0.2) and o (-0.2) columns.
    perm = [1, 0, 3, 2]
    gs = np.array([0.2, 0.2, -0.2, 1.0], np.float32)[None, None, None, :, None]
    Wr = (W_.reshape(3, 3, C, 4, C)[:, :, :, perm, :] * gs).reshape(3, 3, C, 4 * C)
    Ur = (U.reshape(3, 3, C, 4, C)[:, :, :, perm, :] * gs).reshape(3, 3, C, 4 * C)
    # stationary tap j: rows 0:64 = U tap (h half), rows 64:128 = W tap (x);
    # wstk[1] has the kernel rows flipped, for the row-flipped bottom shards
    wstk = np.zeros((2, 9, 128, 256), np.float32)
    for j in range(9):
        a0, b0 = j // 3, j % 3
        wstk[0, j, 0:64] = Ur[a0, b0]
        wstk[0, j, 64:128] = Wr[a0, b0]
        wstk[1, j, 0:64] = Ur[2 - a0, b0]
        wstk[1, j, 64:128] = Wr[2 - a0, b0]
    wstk = np.ascontiguousarray(wstk.transpose(0, 2, 1, 3)).astype(
        ml_dtypes.bfloat16)  # [2, 128, 9, 256]

    b4 = b.reshape(4, C)[perm]  # rows f,i,o,g
    consts = np.zeros((128, 4), np.float32)
    consts[0:64, 0] = 0.2 * b4[0] + 0.5
    consts[64:128, 0] = 0.2 * b4[1] + 0.5
    consts[0:64, 1] = 0.5 - 0.2 * b4[2]
    consts[64:128, 1] = b4[3]
    scale = gamma / np.sqrt(moving_var + BN_EPS)
    consts[0:64, 2] = scale
    consts[0:64, 3] = beta - moving_mean * scale

    zplane = np.zeros((128, RP, CP), ml_dtypes.bfloat16)
    foldw = np.tile(np.eye(64, dtype=np.float32), (2, 1)).astype(
        ml_dtypes.bfloat16)
    in_maps = []
    for k in range(8):
        s, half = k // 2, k % 2
        if half == 0:
            xs = x[s, :, 0:HS]
        else:
            xs = x[s, :, H - HS:H][:, ::-1]  # row-flipped bottom shard
        xs = np.ascontiguousarray(xs.transpose(0, 3, 1, 2)).astype(
            ml_dtypes.bfloat16)
        xp = np.zeros((T, C, HS, CP), ml_dtypes.bfloat16)
        xp[:, :, :, 1:65] = xs
        xs = xp
        in_maps.append({"xc": xs, "wstk": wstk[half], "consts": consts,
                        "zeros": zplane, "foldw": foldw})
    return in_maps


def kernel(input_tensor, W, U, b, gamma, beta, moving_mean, moving_var):
    in_maps = _prep_inputs(input_tensor, W, U, b, gamma, beta,
                           moving_mean, moving_var)
    res = run_bass_kernel_spmd(_get_nc(), in_maps, core_ids=list(range(8)))

    out = np.empty((B, T, 32, 32, C), np.float32)
    for k in range(8):
        s, half = k // 2, k % 2
        yc = res.results[k]["yout"].reshape(T, C, 16, 32).transpose(0, 2, 3, 1)
        if half == 0:
            out[s, :, 0:16] = yc
        else:
            out[s, :, 16:32] = yc[:, ::-1]
    return out
